# revision 38
# baseline (speedup 1.0000x reference)
"""Brownian/OU bridge sampler kernel for Trainium2 (8 NeuronCores).

Problem (per batch element b, time series of length T, DIM=64 channels):
  first 32 channels:  bm = cumsum_t(sqrt(dt)*noise) / (sqrt(t)+1e-8)
  last 32 channels:   ou = e^{-theta t} * cumsum_t(sqrt(e^{2 theta t}-e^{2 theta t'})
                           * sigma/sqrt(2 theta) * noise)
                           / (sigma*sqrt((1-e^{-2 theta t})/(2 theta))+1e-8)

Strategy: pure data parallel over batch (32 samples per core); no cross-core
communication. The rel-err gate (2e-2) leaves a large precision budget, so
noise and the output cross HBM as bf16 (host casts in kernel()), halving the
33.5 MB/core fp32 traffic to 16.8 MB -> ~47 us DMA roofline per core.

Layout: time-QUAD — partition p of a 512-step chunk holds t = 4p+l, l<4, so
every DMA descriptor covers 4 steps x 64 ch x 2 B = 512 B (the SDMA line-rate
knee; smaller descriptors run at half rate). The cumsum INCLUDING the
within-quad positions comes from PSUM-accumulated matmuls per 8-sample
group: two banks per group, bank01 = Li.r0 + Li.r1 + Ls.r2 + Ls.r3 (Li/Ls =
inclusive/strict triangular-ones stationaries over quads) = prefix through
(q, l=1), and bank23 = Li.(r0+r1+r2+r3) = prefix through (q, l=3). This
puts ALL reduction work on the otherwise-idle PE; the DVE keeps only the
r-mul, one merged subtract (a0 = a1 - r1, a2 = a3 - r3), and the norm-mul
— dropping DVE busy from ~64 us to ~45 us/rep, below the 46.6 us DMA floor.
Cross-chunk carries ride a bf16 matmul per bank selecting PSUM row 127 from
an aligned 32-row stash (ACT copy of bank23 rows 96:128). Both banks of a
group live in ONE 2-bank PSUM tile so a single ACT copy (strided over sct
slots 1,3) moves them to SBUF. Store DMAs split per group and alternate
SP/ACT HWDGE queues so a store parked on its mul never head-of-line-blocks
the next slot's loads (SP) or bank copies (ACT); loads prefetch 2 slots
ahead on SP for the same reason.

Per-timestep coefficients are precomputed once on a compact [128, (s,i2,l,h)]
layout (flat full-speed ts load + 4 strided PE-transposes), then EXPANDED
along the channel dim into [P][i2][g][s][l][h][d] bf16 tables. The expansion
is rep-invariant preamble work; it makes every steady-state elementwise op
fully packed bf16, which the DVE runs in its 2x 16-bit mode. All elementwise
work lives on the DVE (HW showed Q7/Pool bf16 tensor ops run ~2x slower than
the cost model, so any Pool op straggles the per-half store); the four
norm-muls issue as ONE instruction over an l-major scratch tile. ACT does
PSUM->SBUF bf16 copies, carry stashes, and the store-DMA ring; loads ride SP.

Numerics: exp(2θt)-exp(2θt') is restructured as exp(2θt')*expm1(2θ dt)
(cubic Taylor expm1; dt<=1e-2) and 1-exp(-2θt) uses a degree-6 Taylor/direct
blend at 2θt=0.5 — the coefficient pipeline stays fp32 until the final bf16
rounding. End-to-end error vs the fp32 reference is ~4e-3 (bf16 quantization
of noise/coefficients/output), comfortably inside the 2e-2 gate.
"""
import numpy as np
import ml_dtypes

import bass_rust
import concourse.bass as bass
import concourse.tile as tile
from concourse import mybir
from concourse.bass_utils import run_bass_kernel_spmd

B, T, DIM = 256, 2048, 64
THETA = 0.1
N_CORES = 8
NB = B // N_CORES      # 32 samples per core
P = 128                # partitions
L4 = 4                 # quad: timesteps per partition per chunk
NC4 = T // (L4 * P)    # 4 time chunks of 512 steps
S = 8                  # samples packed per matmul free dim
G = NB // S            # 4 carry chains per core
HS = 16                # samples per DMA (half chunk)
H = 2                  # halves (bm / ou)
DH = DIM // H          # 32
FREE = S * DIM         # 512 = one PSUM bank of fp32

STORE_ENG = ("sp", "act")  # per-group store DMA queue: "sp" or "act"
LOAD_ENG = ("sp",)         # per-half load DMA queues
STORE_SPLIT = 1            # stores per group (1 or 2)
PF = 2                     # load prefetch depth in slots
# calibration level: 0 = full kernel, 1 = DMA + r-mul + sub + mul (no PE/ACT),
# 2 = DMA only
STRIP = 0
# True = original suffix-sum main loop (for same-session A/B timing)
BASELINE = False
# "two_bank": 2 PSUM banks/group (min DVE, 20 matmuls/slot)
# "one_bank": inclusive bank only + DVE sub-ladder (10 matmuls/slot)
VARIANT = "two_bank"

F32 = mybir.dt.float32
F32R = mybir.dt.float32r
BF16 = mybir.dt.bfloat16
AF = mybir.ActivationFunctionType
OP = mybir.AluOpType


def _split_waits(nc, max_waits=1):
    """walrus in this container rejects >1 sem wait per instruction; hoist
    extras onto same-engine NoOps inserted just before the offender."""
    n = 0
    for f in nc.m.functions:
        for blk in f.blocks:
            insts = blk.instructions
            i = 0
            while i < len(insts):
                inst = insts[i]
                si = inst.sync_info
                if si is not None and len(si.on_wait) > max_waits:
                    waits = list(si.on_wait)
                    keep, rest = waits[:max_waits], waits[max_waits:]
                    nops = []
                    for j in range(0, len(rest), max_waits):
                        nop = bass_rust.InstNoOp(name=f"I-ws-{n}", ins=[], outs=[])
                        n += 1
                        nop.engine = inst.engine
                        nop.sync_info = mybir.SyncInfo(
                            on_wait=rest[j : j + max_waits], on_update=[])
                        nops.append(nop)
                    inst.sync_info = mybir.SyncInfo(
                        on_wait=keep, on_update=list(si.on_update))
                    for k, nop in enumerate(nops):
                        insts.insert(i + k, nop)
                    i += len(nops)
                i += 1
    return nc


def _strided(ap_full, offset_elems, step, count):
    """[P, count] view of a tile's free space at element offset with stride."""
    return bass.AP(
        tensor=ap_full.tensor,
        offset=ap_full.offset + offset_elems,
        ap=[list(ap_full.ap[0]), [step, count]],
    )


def _build(reps: int = 1, hw_loop: int = 0):
    nc = bass.Bass("TRN2")
    ts_in = nc.dram_tensor("ts", [NB, T, 1], F32, kind="ExternalInput")
    nz_in = nc.dram_tensor("noise", [NB, T, DIM], BF16, kind="ExternalInput")
    out = nc.dram_tensor("out", [NB, T, DIM], BF16, kind="ExternalOutput")

    ts_flat = ts_in[:, :, 0].rearrange("s t -> (s t)")

    with tile.TileContext(nc) as tc:
        with (
            tc.tile_pool(name="consts", bufs=1) as consts,
            tc.tile_pool(name="nzp", bufs=4) as nzp,
            tc.tile_pool(name="op_", bufs=2) as op_,
            tc.tile_pool(name="psp", bufs=3, space="PSUM") as psp,
        ):
            # ---------------- constants ----------------
            cw0_cm = tc.tile_pool(name="cw0", bufs=1)
            cw0 = cw0_cm.__enter__()
            ones_t = cw0.tile([P, P], F32)
            nc.vector.memset(ones_t, 1.0)
            L = cw0.tile([P, P], F32)          # L[u, q] = 1 if u <= q
            nc.gpsimd.affine_select(
                out=L, in_=ones_t, pattern=[[1, P]], compare_op=OP.is_ge,
                fill=0.0, base=0, channel_multiplier=-1)
            LB = consts.tile([P, P], BF16)        # bf16 inclusive stationary
            nc.vector.tensor_copy(out=LB, in_=L)
            Ls = cw0.tile([P, P], F32)         # Ls[u, q] = 1 if u < q
            nc.gpsimd.affine_select(
                out=Ls, in_=ones_t, pattern=[[1, P]], compare_op=OP.is_ge,
                fill=0.0, base=-1, channel_multiplier=-1)
            LSB = consts.tile([P, P], BF16)       # bf16 strict stationary
            nc.vector.tensor_copy(out=LSB, in_=Ls)
            e31 = cw0.tile([32, P], F32)       # row 31 ones, else 0
            nc.gpsimd.affine_select(
                out=e31, in_=ones_t[0:32, :], pattern=[[0, P]],
                compare_op=OP.is_equal, fill=0.0, base=-31,
                channel_multiplier=1)
            E31R = consts.tile([32, P], BF16)
            nc.vector.tensor_copy(out=E31R, in_=e31)
            ident = cw0.tile([P, P], F32)      # identity for PE transpose
            nc.gpsimd.affine_select(
                out=ident, in_=ones_t, pattern=[[-1, P]],
                compare_op=OP.is_equal, fill=0.0, base=0,
                channel_multiplier=1)

            # expanded per-timestep coefficient tables (rep-invariant):
            # [P][i2][g][s][l][h][d], innermost d packed so every steady-state
            # elementwise op qualifies for the DVE 16-bit fast path.
            cmulx = consts.tile([P, NC4, G, S, L4, H, DH], BF16)
            cnormx = consts.tile([P, NC4, G, S, L4, H, DH], BF16)
            ctmp = [consts.tile([32, FREE], BF16, tag=f"ctmp{g}", name=f"ctmp{g}")
                    for g in range(G)]

            # -------- compact coefficients: [P, (s, i2, l, h)] --------
            # flat time index f = s*2048 + 512*i2 + 4p + l = q*512 + m with
            # q = 4s + i2 (partition of the flat load), m = 4p + l; the four
            # strided PE-transposes T_l[p, q] land at n = 4q + l.
            with (
                tc.tile_pool(name="cw", bufs=1) as cw,
                tc.tile_pool(name="trps", bufs=1, space="PSUM") as trps,
                tc.tile_pool(name="cwp", bufs=1, space="PSUM") as cwp,
                nc.allow_low_precision(
                    reason="coefficients are rounded to bf16 by design; "
                           "all math stays fp32 until the final store"),
            ):
                s1 = cw.tile([P, 512], F32)       # flat[q*512 + m]
                nc.sync.dma_start(
                    out=s1, in_=ts_flat.rearrange("(p f) -> p f", p=P))
                s1p = cw.tile([P, 512], F32)      # flat[q*512 + m - 1]
                nc.sync.dma_start(
                    out=s1p[1:P, :],
                    in_=bass.AP(tensor=ts_flat.tensor,
                                offset=ts_flat.offset + 511,
                                ap=[[512, P - 1], [1, 512]]))
                nc.sync.dma_start(
                    out=s1p[0:1, 1:512],
                    in_=bass.AP(tensor=ts_flat.tensor, offset=ts_flat.offset,
                                ap=[[0, 1], [1, 511]]))
                nc.vector.memset(s1p[0:1, 0:1], 0.0)

                ts_c = cw.tile([P, 32, NC4, L4], F32)   # (s, i2, l)
                tsp_c = cw.tile([P, 32, NC4, L4], F32)
                tsf = ts_c[:, :, :, :].rearrange("p s i l -> p (s i l)")
                tspf = tsp_c[:, :, :, :].rearrange("p s i l -> p (s i l)")
                for src, dstf in ((s1, tsf), (s1p, tspf)):
                    for lv in range(L4):
                        pst = trps.tile([P, P], F32, tag="trp",
                                        name=f"trp{lv}")
                        nc.tensor.transpose(
                            out=pst,
                            in_=_strided(src[:, :], lv, L4, P),
                            identity=ident)
                        nc.vector.tensor_copy(
                            out=_strided(dstf, lv, L4, P), in_=pst)
                # each sample's t=0 has predecessor time 0
                nc.vector.memset(tsp_c[:1, :, 0:1, 0:1], 0.0)

                cmul = cw.tile([P, 32, NC4, L4, H], BF16)
                cnorm = cw.tile([P, 32, NC4, L4, H], BF16)
                cmf = cmul[:, :, :, :, :].rearrange("p s i l h -> p (s i l h)")
                cnf = cnorm[:, :, :, :, :].rearrange("p s i l h -> p (s i l h)")
                NF = 32 * NC4 * L4  # 512
                cm0 = _strided(cmf, 0, 2, NF)
                cm1 = _strided(cmf, 1, 2, NF)
                cn0 = _strided(cnf, 0, 2, NF)
                cn1 = _strided(cnf, 1, 2, NF)

                t0 = cw.tile([P, NF], F32, tag="t0")
                t1 = cw.tile([P, NF], F32, tag="t1")
                t2 = cwp.tile([P, NF], F32, tag="t2")
                t3 = s1       # transpose sources are dead here; reuse
                t4 = s1p

                # db = sqrt(ts - tsp)  (fp32 subtraction is exact here)
                nc.vector.tensor_tensor(out=t0, in0=tsf, in1=tspf,
                                        op=OP.subtract)
                nc.scalar.activation(out=cm0, in_=t0, func=AF.Sqrt)
                # dou = sqrt(5 * exp(.2 tsp) * expm1(.2 (ts-tsp)))
                nc.vector.tensor_scalar_mul(out=t1, in0=t0, scalar1=0.2)
                nc.vector.tensor_scalar(out=t2, in0=t1, scalar1=1.0 / 3.0,
                                        scalar2=1.0, op0=OP.mult, op1=OP.add)
                nc.vector.tensor_mul(out=t3, in0=t1, in1=t2)
                nc.vector.tensor_scalar(out=t2, in0=t3, scalar1=0.5,
                                        scalar2=1.0, op0=OP.mult, op1=OP.add)
                nc.vector.tensor_mul(out=t3, in0=t1, in1=t2)      # expm1
                nc.scalar.activation(out=t2, in_=tspf, func=AF.Exp, scale=0.2)
                nc.vector.tensor_mul(out=t3, in0=t3, in1=t2)
                nc.scalar.activation(out=cm1, in_=t3, func=AF.Sqrt, scale=5.0)
                # nb = 1/(sqrt(ts)+1e-8)
                nc.scalar.activation(out=t0, in_=tsf, func=AF.Sqrt)
                nc.vector.tensor_scalar_add(out=t0, in0=t0, scalar1=1e-8)
                nc.vector.reciprocal(out=cn0, in_=t0)
                # f2 = exp(-.1 ts) / (sqrt(5*(1-exp(-.2 ts))) + 1e-8)
                #   1-exp(-y), y = .2 ts: Taylor (deg 6) below y=0.5 else direct
                nc.vector.tensor_scalar_mul(out=t0, in0=tsf, scalar1=0.2)
                nc.scalar.activation(out=t1, in_=tsf, func=AF.Exp, scale=-0.2)
                nc.vector.tensor_scalar(out=t1, in0=t1, scalar1=-1.0,
                                        scalar2=1.0, op0=OP.mult, op1=OP.add)
                nc.vector.tensor_scalar(out=t2, in0=t0, scalar1=-1.0 / 6.0,
                                        scalar2=1.0, op0=OP.mult, op1=OP.add)
                for k in (5, 4, 3, 2):
                    nc.vector.tensor_mul(out=t3, in0=t0, in1=t2)
                    nc.vector.tensor_scalar(out=t2, in0=t3, scalar1=-1.0 / k,
                                            scalar2=1.0, op0=OP.mult,
                                            op1=OP.add)
                nc.vector.tensor_mul(out=t3, in0=t0, in1=t2)      # taylor
                nc.vector.tensor_scalar(out=t4, in0=t0, scalar1=0.5,
                                        scalar2=None, op0=OP.is_lt)
                nc.vector.tensor_tensor(out=t3, in0=t3, in1=t1,
                                        op=OP.subtract)
                nc.vector.tensor_mul(out=t3, in0=t4, in1=t3)
                nc.vector.tensor_tensor(out=t3, in0=t3, in1=t1, op=OP.add)
                nc.scalar.activation(out=t3, in_=t3, func=AF.Sqrt, scale=5.0)
                nc.vector.tensor_scalar_add(out=t3, in0=t3, scalar1=1e-8)
                nc.vector.reciprocal(out=t3, in_=t3)
                nc.scalar.activation(out=t0, in_=tsf, func=AF.Exp, scale=-0.1)
                nc.vector.tensor_mul(out=cn1, in0=t0, in1=t3)

                # -------- expand along d into the packed bf16 tables --------
                for comp, xt in ((cmul, cmulx), (cnorm, cnormx)):
                    src = comp[:, :, :, :, :].rearrange(
                        "p (g s) i l h -> p i g s l h", g=G)
                    nc.vector.tensor_copy(
                        out=xt[:, :, :, :, :, :, :],
                        in_=src.to_broadcast([P, NC4, G, S, L4, H, DH]))

            cw0_cm.__exit__(None, None, None)
            # scratch opens after the preamble workspace frees its SBUF, so
            # its 3 generations fit without raising the preamble peak.
            scr_cm = tc.tile_pool(name="scr", bufs=3)
            scr = scr_cm.__enter__()
            # ---------------- main scan ----------------
            # Software-pipelined with a one-half skew: the DVE is in-order,
            # so half k's subs (which wait on the ACT PSUM->SBUF copy) are
            # emitted AFTER half k+1's independent r-mul/tree — the DVE never
            # idles on the matmul->ACT->sem latency.
            def emit_load(i2, hf):
                tsl = slice(i2 * L4 * P, (i2 + 1) * L4 * P)
                sh = slice(hf * HS, (hf + 1) * HS)
                nt = nzp.tile([P, HS, L4, DIM], BF16, tag="nz", name="nt")
                # split across both HWDGE queues for DMA-engine parallelism
                for l2, eng in enumerate(LOAD_ENG):
                    w = HS // len(LOAD_ENG)
                    sh_l = slice(sh.start + l2 * w, sh.start + (l2 + 1) * w)
                    e = nc.sync if eng == "sp" else nc.scalar
                    e.dma_start(
                        out=nt[:, l2 * w:(l2 + 1) * w, :, :],
                        in_=nz_in[sh_l, tsl, :].rearrange(
                            "s (p l) d -> p s l d", l=L4))
                return nt

            def emit_front(i2, hf, nt):
                tsl = slice(i2 * L4 * P, (i2 + 1) * L4 * P)
                sh = slice(hf * HS, (hf + 1) * HS)
                gp = slice(hf * (HS // S), hf * (HS // S) + HS // S)
                # r = noise * cmul, in place over the load tile; elementwise
                # runs at 16-sample granularity, matmuls per 8-sample bank.
                ntv = nt[:, :, :, :].rearrange(
                    "p (g s) l (h d) -> p g s l h d", g=HS // S, h=H)
                if STRIP < 2:
                    nc.vector.tensor_mul(
                        out=ntv, in0=ntv, in1=cmulx[:, i2, gp, :, :, :, :])
                # Within-quad prefixes come from TWO PSUM banks per group,
                # each accumulating matmuls over the r_l slices directly:
                #   bank01[q] = carry + Li.r0 + Li.r1 + Ls.r2 + Ls.r3
                #             = prefix through (q, l=1)
                #   bank23[q] = carry + Li.(r0+r1+r2+r3)
                #             = prefix through (q, l=3)
                # (Li inclusive, Ls strict over quads.)  This moves the three
                # suffix adds off the DVE onto the idle PE; the DVE keeps one
                # merged subtract (slots 0/2) and the final norm-mul.
                # Slot map: 0 = a0, 1 = a1 (ACT), 2 = a2, 3 = a3 (ACT).
                sct = scr.tile([P, L4, HS, DIM], BF16, tag="sct")
                for g2 in range(HS // S if STRIP == 0 else 0):
                    g = hf * (HS // S) + g2
                    sgs = slice(g2 * S, (g2 + 1) * S)
                    rl = [nt[:, sgs, lv, :] for lv in range(L4)]
                    if VARIANT == "one_bank":
                        ps23 = psp.tile([P, FREE], F32, tag="psb",
                                        name="ps23")
                        for lv in range(L4):
                            nc.tensor.matmul(
                                ps23, lhsT=LB, rhs=rl[lv], start=(lv == 0),
                                stop=(lv == 3 and i2 == 0))
                        if i2 > 0:
                            nc.tensor.matmul(ps23, lhsT=E31R, rhs=ctmp[g],
                                             start=False, stop=True)
                        if i2 < NC4 - 1:
                            nc.scalar.activation(
                                out=ctmp[g], in_=ps23[96:128, :],
                                func=AF.Copy)
                        nc.scalar.activation(
                            out=sct[:, 3, sgs, :],
                            in_=ps23[:, :].rearrange(
                                "p (s d) -> p s d", s=S),
                            func=AF.Copy)
                        continue
                    psb = psp.tile([P, 2, FREE], F32, tag="psb")
                    ps01 = psb[:, 0, :]
                    ps23 = psb[:, 1, :]
                    # bank01 complete first so its ACT copy fires early
                    nc.tensor.matmul(ps01, lhsT=LB, rhs=rl[0],
                                     start=True, stop=False)
                    nc.tensor.matmul(ps01, lhsT=LB, rhs=rl[1],
                                     start=False, stop=False)
                    nc.tensor.matmul(ps01, lhsT=LSB, rhs=rl[2],
                                     start=False, stop=False)
                    nc.tensor.matmul(ps01, lhsT=LSB, rhs=rl[3],
                                     start=False, stop=(i2 == 0))
                    if i2 > 0:
                        nc.tensor.matmul(ps01, lhsT=E31R, rhs=ctmp[g],
                                         start=False, stop=True)
                    nc.tensor.matmul(ps23, lhsT=LB, rhs=rl[0],
                                     start=True, stop=False)
                    nc.tensor.matmul(ps23, lhsT=LB, rhs=rl[1],
                                     start=False, stop=False)
                    nc.tensor.matmul(ps23, lhsT=LB, rhs=rl[2],
                                     start=False, stop=False)
                    nc.tensor.matmul(ps23, lhsT=LB, rhs=rl[3],
                                     start=False, stop=(i2 == 0))
                    if i2 > 0:
                        nc.tensor.matmul(ps23, lhsT=E31R, rhs=ctmp[g],
                                         start=False, stop=True)
                    if i2 < NC4 - 1:
                        nc.scalar.activation(
                            out=ctmp[g], in_=ps23[96:128, :], func=AF.Copy)
                    # one copy fills sct slots 1 AND 3 from the 2-bank tile
                    nc.scalar.activation(
                        out=sct[:, :, sgs, :].rearrange(
                            "p (a b) s d -> p b a s d", b=2)[:, 1, :, :, :],
                        in_=psb[:, :, :].rearrange(
                            "p b (s d) -> p b s d", s=S),
                        func=AF.Copy)
                return dict(i2=i2, hf=hf, tsl=tsl, sh=sh, nt=nt, sct=sct,
                            gp=gp)

            def emit_back_g(st, g2):
                i2, hf, nt, sct = st["i2"], st["hf"], st["nt"], st["sct"]
                g = hf * (HS // S) + g2
                sgs = slice(g2 * S, (g2 + 1) * S)
                if g2 == 0 and STRIP < 2:
                    st["o"] = op_.tile([P, HS, L4, DIM], BF16, tag="o",
                                       name="o")
                o = st.get("o")
                # a0 = a1 - r1, a2 = a3 - r3 in ONE merged instruction over
                # the even slots (l stride 2 on out/in0; r slices 1,3 on in1)
                sctv = sct[:, :, :, :].rearrange(
                    "p (a b) s d -> p b a s d", b=2)
                ntv2 = nt[:, :, :, :].rearrange(
                    "p s (a b) d -> p b a s d", b=2)
                if STRIP == 0 and VARIANT == "one_bank":
                    if g2 == 0:
                        # chained sub-ladder per half: a2 = a3-r3, a1 = a2-r2,
                        # a0 = a1-r1 (full-half ops, emitted once at g2==0)
                        for lv in (2, 1, 0):
                            nc.vector.tensor_tensor(
                                out=sct[:, lv, :, :],
                                in0=sct[:, lv + 1, :, :],
                                in1=nt[:, :, lv + 1, :],
                                op=OP.subtract)
                elif STRIP == 0:
                    nc.vector.tensor_tensor(
                        out=sctv[:, 0, :, sgs, :],
                        in0=sctv[:, 1, :, sgs, :],
                        in1=ntv2[:, 1, :, sgs, :],
                        op=OP.subtract)
                # out_l = a_l * n_l for all four l in ONE packed instruction
                if STRIP < 2:
                    src = (sct[:, :, sgs, :] if STRIP == 0 else
                           nt[:, :, :, :].rearrange(
                               "p s l d -> p l s d")[:, :, sgs, :])
                    nc.vector.tensor_mul(
                        out=o[:, sgs, :, :].rearrange(
                            "p s l (h d) -> p l s h d", h=H),
                        in0=src.rearrange("p l s (h d) -> p l s h d", h=H),
                        in1=cnormx[:, i2, g, :, :, :, :].rearrange(
                            "p s l h d -> p l s h d"))
                src_store = (o if STRIP < 2 else nt)
                w = S // STORE_SPLIT
                for q2 in range(STORE_SPLIT):
                    j = g2 * STORE_SPLIT + q2
                    s0 = g2 * S + q2 * w
                    sh_g = slice(st["sh"].start + s0,
                                 st["sh"].start + s0 + w)
                    eng = (nc.sync if STORE_ENG[j % len(STORE_ENG)] == "sp"
                           else nc.scalar)
                    eng.dma_start(
                        out=out[sh_g, st["tsl"], :].rearrange(
                            "s (p l) d -> p s l d", l=L4),
                        in_=src_store[:, s0:s0 + w, :, :])

            def emit_front_base(i2, hf, nt):
                tsl = slice(i2 * L4 * P, (i2 + 1) * L4 * P)
                sh = slice(hf * HS, (hf + 1) * HS)
                gp = slice(hf * (HS // S), hf * (HS // S) + HS // S)
                ntv = nt[:, :, :, :].rearrange(
                    "p (g s) l (h d) -> p g s l h d", g=HS // S, h=H)
                nc.vector.tensor_mul(
                    out=ntv, in0=ntv, in1=cmulx[:, i2, gp, :, :, :, :])
                sct = scr.tile([P, L4, HS, DIM], BF16, tag="sct")
                nc.vector.tensor_tensor(
                    out=sct[:, 1, :, :], in0=nt[:, :, 3, :],
                    in1=nt[:, :, 2, :], op=OP.add)
                nc.vector.tensor_tensor(
                    out=sct[:, 0, :, :], in0=sct[:, 1, :, :],
                    in1=nt[:, :, 1, :], op=OP.add)
                nc.vector.tensor_tensor(
                    out=sct[:, 2, :, :], in0=sct[:, 0, :, :],
                    in1=nt[:, :, 0, :], op=OP.add)
                for g2 in range(HS // S):
                    g = hf * (HS // S) + g2
                    sgs = slice(g2 * S, (g2 + 1) * S)
                    ps = psp.tile([P, FREE], F32, tag="psb", name="ps")
                    nc.tensor.matmul(
                        ps, lhsT=LB, rhs=sct[:, 2, sgs, :],
                        start=True, stop=(i2 == 0))
                    if i2 > 0:
                        nc.tensor.matmul(ps, lhsT=E31R, rhs=ctmp[g],
                                         start=False, stop=True)
                    if i2 < NC4 - 1:
                        nc.scalar.activation(
                            out=ctmp[g], in_=ps[96:128, :], func=AF.Copy)
                    nc.scalar.activation(
                        out=sct[:, 3, sgs, :],
                        in_=ps[:, :].rearrange("p (s d) -> p s d", s=S),
                        func=AF.Copy)
                return dict(i2=i2, hf=hf, tsl=tsl, sh=sh, nt=nt, sct=sct,
                            gp=gp)

            def emit_back_base(st):
                i2, gp, nt, sct = st["i2"], st["gp"], st["nt"], st["sct"]
                o = op_.tile([P, HS, L4, DIM], BF16, tag="o", name="o")
                sb3 = sct[:, 3, :, :]
                nc.vector.tensor_tensor(
                    out=sct[:, 0, :, :], in0=sb3,
                    in1=sct[:, 0, :, :], op=OP.subtract)
                nc.vector.tensor_tensor(
                    out=sct[:, 1, :, :], in0=sb3,
                    in1=sct[:, 1, :, :], op=OP.subtract)
                nc.vector.tensor_tensor(
                    out=sct[:, 2, :, :], in0=sb3,
                    in1=nt[:, :, 3, :], op=OP.subtract)
                nc.vector.tensor_mul(
                    out=o[:, :, :, :].rearrange(
                        "p s l (h d) -> p l s h d", h=H),
                    in0=sct[:, :, :, :].rearrange(
                        "p l s (h d) -> p l s h d", h=H),
                    in1=cnormx[:, st["i2"], gp, :, :, :, :].rearrange(
                        "p g s l h d -> p l (g s) h d"))
                nc.scalar.dma_start(
                    out=out[st["sh"], st["tsl"], :].rearrange(
                        "s (p l) d -> p s l d", l=L4),
                    in_=o)

            def emit_reps_base():
                pend = None
                slots = [(i2, hf)
                         for _rep in range(reps)
                         for i2 in range(NC4)
                         for hf in range(NB // HS)]
                for i2, hf in slots:
                    nt = emit_load(i2, hf)
                    st = emit_front_base(i2, hf, nt)
                    if pend is not None:
                        emit_back_base(pend)
                    pend = st
                if pend is not None:
                    emit_back_base(pend)

            def emit_reps():
                # loads run PF slots ahead of compute so a store (waiting on
                # its mul) parked at the SP queue head never starves a load.
                slots = [(i2, hf)
                         for _rep in range(reps)
                         for i2 in range(NC4)
                         for hf in range(NB // HS)]
                loads = [emit_load(*slots[k]) for k in range(min(PF, len(slots)))]
                pend = None
                for k, (i2, hf) in enumerate(slots):
                    if k + PF < len(slots):
                        loads.append(emit_load(*slots[k + PF]))
                    st = emit_front(i2, hf, loads[k])
                    if pend is not None:
                        for g2 in range(HS // S):
                            emit_back_g(pend, g2)
                    pend = st
                if pend is not None:
                    for g2 in range(HS // S):
                        emit_back_g(pend, g2)

            emitter = emit_reps_base if BASELINE else emit_reps
            if hw_loop:
                with tc.For_i(0, hw_loop):
                    emitter()
            else:
                emitter()
            scr_cm.__exit__(None, None, None)
    _split_waits(nc)
    return nc


_NC = None


def _get_nc():
    global _NC
    if _NC is None:
        _NC = _build()
    return _NC


def kernel(ts: np.ndarray, noise: np.ndarray) -> np.ndarray:
    ts = np.ascontiguousarray(ts, dtype=np.float32)
    noise_bf = np.ascontiguousarray(noise, dtype=np.float32).astype(
        ml_dtypes.bfloat16)
    in_maps = [
        {"ts": ts[c * NB : (c + 1) * NB],
         "noise": noise_bf[c * NB : (c + 1) * NB]}
        for c in range(N_CORES)
    ]
    res = run_bass_kernel_spmd(_get_nc(), in_maps, core_ids=list(range(N_CORES)))
    return np.concatenate(
        [r["out"].astype(np.float32) for r in res.results], axis=0)

